# revision 20
# baseline (speedup 1.0000x reference)
"""Trainium2 Bass kernel for nn_BinaryLinear (binarized 4-layer MLP + BatchNorm).

Reference computation (fp32, jax):
    h = x.reshape(-1, 3072)
    h = relu(h @ sign(W1).T); h = BN(h, g1, b1)   # BN over full 8192 batch
    h = relu(h @ sign(W2).T); h = BN(h, g2, b2)
    h = relu(h @ sign(W3).T); h = BN(h, g3, b3)
    out = h @ sign(W4).T                          # [8192, 10]

Strategy (8 NeuronCores, data-parallel over batch):
  - Host: binarize weights to bf16 (+-1 exact), pack everything partition-
    major ([128, ktiles, free]), shard x over cores (1024 rows each).
  - Device: activations feature-major [feature_part, batch_free] in SBUF.
    Each layer is K-tiled bf16 matmuls accumulating in PSUM, feature tiles
    processed in chain-groups [0-3 k-outer], [4], [5,6], [7 half-split].
  - BatchNorm stats (sum, sumsq of relu) exchanged in TWO AllReduces per
    layer: tiles {0..3} (ready ~40% into the layer; scale/shift applied
    well before the layer ends) and tiles {4..7} (ready at layer end,
    resolving under the next layer's k=0..3 matmuls, which only need the
    first AllReduce's tiles). Stats DMAs ride the gpsimd queue (with the
    collective triggers) and readbacks ride the vector queue so they never
    head-of-line-block the weight feeds (sync/scalar rings).
  - Warmup: ~10 matmuls on zeroed scratch warm the PE clock (HAM) while the
    first XT/W1 chunks stream in. Layer 1 splits its early stats into two
    small AllGathers ({0,1} at ~40us, {2,3} at ~68us) so the first
    collective's ncfw wake + peer spread resolve with huge margin.
"""
import os
import sys

for _p in ("/opt/trn_rl_repo",):
    if os.path.isdir(_p) and _p not in sys.path:
        sys.path.insert(0, _p)

import numpy as np
import ml_dtypes

from concourse import bacc, tile, mybir
from concourse import bass_utils

NCORES = 8
B = 8192
BL = B // NCORES            # 1024 rows per core
KIN = 3072
KT_IN = KIN // 128          # 24 k-tiles for layer 1
HID = 1024
JT = HID // 128             # 8 feature tiles
CLS = 10
CLSP = 16                   # padded classes
EPS = 1e-5
BF16 = mybir.dt.bfloat16
F32 = mybir.dt.float32
ADD = mybir.AluOpType.add
SUB = mybir.AluOpType.subtract
MUL = mybir.AluOpType.mult
BYP = mybir.AluOpType.bypass
RELU = mybir.ActivationFunctionType.Relu

_CACHE = {}

# feature-tile groups for the two per-layer stats AllReduces
G1 = [0, 1, 2, 3]
G2 = [4, 5, 6, 7]
HALVES = [(0, 512), (512, 512)]


def _build():
    nc = bacc.Bacc("TRN2", target_bir_lowering=False, debug=False, num_devices=NCORES)

    xt_d = nc.dram_tensor("xt", [128, KT_IN, BL], BF16, kind="ExternalInput")
    w1_d = nc.dram_tensor("w1t", [128, KT_IN, HID], BF16, kind="ExternalInput")
    w2_d = nc.dram_tensor("w2t", [128, JT, HID], BF16, kind="ExternalInput")
    w3_d = nc.dram_tensor("w3t", [128, JT, HID], BF16, kind="ExternalInput")
    w4_d = nc.dram_tensor("w4t", [128, JT, CLSP], BF16, kind="ExternalInput")
    bnp_d = nc.dram_tensor("bnp", [128, 6 * JT], F32, kind="ExternalInput")
    out_d = nc.dram_tensor("out", [CLSP, BL], F32, kind="ExternalOutput")

    with tile.TileContext(nc) as tc:
        with (
            tc.tile_pool(name="weights", bufs=1) as wpool,
            tc.tile_pool(name="acts", bufs=1) as apool,
            tc.tile_pool(name="scratch", bufs=2) as scrpool,
            tc.tile_pool(name="stats", bufs=2) as spool,
            tc.tile_pool(name="psum", bufs=4, space="PSUM") as pspool,
            tc.tile_pool(name="dram", bufs=2, space="DRAM") as dpool,
        ):
            XT = wpool.tile([128, KT_IN, BL], BF16, tag="XT")
            W1 = wpool.tile([128, KT_IN, HID], BF16, tag="W1")
            W2 = wpool.tile([128, JT, HID], BF16, tag="W2")
            W3 = wpool.tile([128, JT, HID], BF16, tag="W3")
            W4 = wpool.tile([128, JT, CLSP], BF16, tag="W4")
            BNP = wpool.tile([128, 6 * JT], F32, tag="BNP")
            HRAW = apool.tile([128, JT, BL], BF16, tag="HRAW")
            H = apool.tile([128, JT, BL], BF16, tag="H")
            H2 = apool.tile([128, JT, BL], BF16, tag="H2")
            WZ = wpool.tile([128, 512], BF16, tag="WZ")

            # ---- warmup ----
            # Wake-absorbing collective FIRST: without it the first real
            # AllGather pays each core's ncfw wake as 20-25us of "peer
            # spread" that echoes through every later mesh. Its result is
            # consumed only by a DCE anchor at the very end of the program.
            wu_in = dpool.tile([128, 1], F32, tag="wu_in")
            wu_out = dpool.tile([NCORES * 128, 1], F32, tag="wu_out")
            nc.gpsimd.collective_compute(
                "AllGather", BYP,
                replica_groups=[list(range(NCORES))],
                ins=[wu_in.opt()], outs=[wu_out.opt()],
            )
            nc.gpsimd.memset(WZ[:], 0)
            # PE-clock warmup: ~8 N=512 matmuls on zeroed scratch keep the PE
            # busy while the first XT/W1 chunks land, so the real stream
            # starts at full clock instead of paying the HAM cold window.
            wps = pspool.tile([128, BL], F32, tag="ps", name="ps_warm")
            for i in range(10):
                mi = nc.tensor.matmul(
                    wps[:, 0:512], WZ[:, 0:128], WZ[:, 0:512],
                    start=True, stop=True,
                )
                if i > 0:
                    mi.ins.ldweights = False

            # ---- input feed ----
            # XT on the sync HWDGE ring, all weights on the scalar ring;
            # first chunks are single k-tiles so the first matmul starts
            # early. The sync ring drains by ~44us and then carries ONLY the
            # BN stats traffic (cc_in writes + gather reads) + final output,
            # so stats DMAs never queue behind megabytes of feed.
            feed = [1, 1] + [2] * 11
            c = 0
            for w in feed:
                nc.sync.dma_start(XT[:, c : c + w, :], xt_d[:, c : c + w, :])
                nc.scalar.dma_start(W1[:, c : c + w, :], w1_d[:, c : c + w, :])
                c += w
            assert c == KT_IN
            nc.sync.dma_start(BNP[:], bnp_d[:])
            nc.scalar.dma_start(W2[:, 0:4, :], w2_d[:, 0:4, :])
            nc.scalar.dma_start(W2[:, 4:8, :], w2_d[:, 4:8, :])
            nc.scalar.dma_start(W3[:, 0:4, :], w3_d[:, 0:4, :])
            nc.scalar.dma_start(W3[:, 4:8, :], w3_d[:, 4:8, :])
            nc.scalar.dma_start(W4[:], w4_d[:])

            def mm_pair(ps, Wk, rhs, k, start, stop):
                for idx, (s, w) in enumerate(HALVES):
                    mi = nc.tensor.matmul(
                        ps[:, s : s + w], Wk, rhs[:, k, s : s + w],
                        start=start, stop=stop,
                    )
                    if idx > 0:
                        mi.ins.ldweights = False

            def relu_tile(ps, jt, S, col):
                nc.scalar.activation(
                    HRAW[:, jt, :], ps[:], RELU, accum_out=S[:, col : col + 1]
                )

            def sq_tile(jt, S, col, s=0, w=BL):
                scr = scrpool.tile([128, w], BF16, tag="scr", name=f"scr_{jt}_{s}")
                nc.vector.scalar_tensor_tensor(
                    scr[:], HRAW[:, jt, s : s + w], 0.0, HRAW[:, jt, s : s + w],
                    BYP, MUL, accum_out=S[:, col : col + 1],
                )

            def ar_start(li, gi, S, n2):
                """stats DMA (sync ring -- idle after the feeds) + AllGather
                trigger (gpsimd queue carries ONLY triggers: any DMA wait
                there would stall the collectives' own data movement)."""
                cc_in = dpool.tile([128, n2], F32, tag="cc_in", name=f"cc_in_{li}_{gi}")
                cc_out = dpool.tile(
                    [NCORES * 128, n2], F32, tag="cc_out", name=f"cc_out_{li}_{gi}"
                )
                nc.sync.dma_start(cc_in[:], S[:])
                nc.gpsimd.collective_compute(
                    "AllGather", BYP,
                    replica_groups=[list(range(NCORES))],
                    ins=[cc_in.opt()], outs=[cc_out.opt()],
                )
                return cc_out

            def ar_finish(li, gi, jts, cc_out):
                """readback (scalar ring) + tree-reduce and scale/shift math
                (vector); returns (A, C)."""
                n = len(jts)
                n2 = 2 * n
                GAT = spool.tile([128, NCORES, n2], F32, tag="GAT",
                                 name=f"GAT_{li}_{gi}")
                nc.sync.dma_start(
                    GAT[:], cc_out.opt().rearrange("(c p) s -> p c s", p=128)
                )
                T4 = spool.tile([128, 4, n2], F32, tag="T4", name=f"T4_{li}_{gi}")
                nc.vector.tensor_tensor(T4[:], GAT[:, 0:4, :], GAT[:, 4:8, :], ADD)
                T2 = spool.tile([128, 2, n2], F32, tag="T2", name=f"T2_{li}_{gi}")
                nc.vector.tensor_tensor(T2[:], T4[:, 0:2, :], T4[:, 2:4, :], ADD)
                RS = spool.tile([128, n2], F32, tag="RS", name=f"RS_{li}_{gi}")
                nc.vector.tensor_tensor(RS[:], T2[:, 0, :], T2[:, 1, :], ADD)
                MEAN = spool.tile([128, n], F32, tag="MEAN", name=f"MEAN_{li}_{gi}")
                nc.vector.tensor_scalar_mul(MEAN[:], RS[:, 0:n], 1.0 / B)
                VPE = spool.tile([128, n], F32, tag="VPE", name=f"VPE_{li}_{gi}")
                nc.vector.tensor_scalar(
                    VPE[:], RS[:, n : 2 * n], 1.0 / B, EPS, MUL, ADD
                )
                MSQ = spool.tile([128, n], F32, tag="MSQ", name=f"MSQ_{li}_{gi}")
                nc.vector.tensor_tensor(MSQ[:], MEAN[:], MEAN[:], MUL)
                VAR = spool.tile([128, n], F32, tag="VAR", name=f"VAR_{li}_{gi}")
                nc.vector.tensor_tensor(VAR[:], VPE[:], MSQ[:], SUB)
                RINV = spool.tile([128, n], F32, tag="RINV", name=f"RINV_{li}_{gi}")
                nc.vector.reciprocal(RINV[:], VAR[:])
                RSTD = spool.tile([128, n], F32, tag="RSTD", name=f"RSTD_{li}_{gi}")
                nc.scalar.sqrt(RSTD[:], RINV[:])
                g0 = (2 * li) * JT + jts[0]
                b0 = (2 * li + 1) * JT + jts[0]
                A = spool.tile([128, n], F32, tag="A", name=f"A_{li}_{gi}")
                nc.vector.tensor_tensor(A[:], RSTD[:], BNP[:, g0 : g0 + n], MUL)
                AM = spool.tile([128, n], F32, tag="AM", name=f"AM_{li}_{gi}")
                nc.vector.tensor_tensor(AM[:], A[:], MEAN[:], MUL)
                C = spool.tile([128, n], F32, tag="C", name=f"C_{li}_{gi}")
                nc.vector.tensor_tensor(C[:], BNP[:, b0 : b0 + n], AM[:], SUB)
                return A, C

            def apply_tile(Hdst, jt, A, C, jj):
                nc.vector.tensor_scalar(
                    Hdst[:, jt, :], HRAW[:, jt, :],
                    A[:, jj : jj + 1], C[:, jj : jj + 1], MUL, ADD,
                )

            def chain_group(jts, kt, rhs, W, li):
                """k-outer accumulation chains for the given feature tiles."""
                pss = [
                    pspool.tile([128, BL], F32, tag="ps", name=f"ps{li}_{j}")
                    for j in jts
                ]
                for k in range(kt):
                    for ps, j in zip(pss, jts):
                        mm_pair(ps, W[:, k, j * 128 : (j + 1) * 128],
                                rhs, k, k == 0, k == kt - 1)
                return pss

            def mlp_layer(li, kt, rhs, W, Hdst):
                """One layer. Layer 1 chains [0,1,2],[3,4] (the 3-chain lead
                group consumes k-tiles slower than the XT/W1 feed delivers,
                and its stats AllGather at ~55us absorbs the CC wake + peer
                spread far from any deadline); layers 2/3 chain [0-3].
                Then [4],[5,6],[7 halves] (mids), with the final stats group
                firing at layer end and resolving under the next layer's
                early k-tile matmuls. Vector-queue order keeps the mid
                tiles' sq ahead of the early groups' tree/math so a slow
                early collective never delays the final stats path."""
                if li == 0:
                    early, mids = [[0, 1, 2], [3, 4, 5]], [[6]]
                else:
                    early, mids = [[0, 1, 2, 3], [4, 5]], [[6]]
                finalj = list(range(early[-1][-1] + 1, JT))
                nf = len(finalj)
                S2 = spool.tile([128, 2 * nf], F32, tag="S2", name=f"S2_{li}")
                Sh = spool.tile([128, 4], F32, tag="Sh", name=f"Sh_{li}")

                ccs = []
                for gi, jts in enumerate(early):
                    n = len(jts)
                    Sg = spool.tile([128, 2 * n], F32, tag=f"S1_{gi}",
                                    name=f"S1_{li}_{gi}")
                    pss = chain_group(jts, kt, rhs, W, li)
                    for i, (ps, j) in enumerate(zip(pss, jts)):
                        relu_tile(ps, j, Sg, i)
                        sq_tile(j, Sg, n + i)
                    ccs.append((jts, Sg, ar_start(li, gi, Sg, 2 * n)))

                # mid chain-groups feed the final stats block
                col = 0
                for jts in mids:
                    pss = chain_group(jts, kt, rhs, W, li)
                    for ps, j in zip(pss, jts):
                        relu_tile(ps, j, S2, col)
                        sq_tile(j, S2, nf + col)
                        col += 1

                # early groups resolve + apply (vector: after the mid sq's
                # so AllGather latency can't delay the final stats path)
                for gi, (jts, Sg, cc) in enumerate(ccs):
                    Ag, Cg = ar_finish(li, gi, jts, cc)
                    for jj, jt in enumerate(jts):
                        apply_tile(Hdst, jt, Ag, Cg, jj)

                # tile 7: two half-batch chains sharing one psum tile, so
                # the first half's relu/sq start before the second half ends
                ps7 = pspool.tile([128, BL], F32, tag="ps", name=f"ps7_{li}")
                for k in range(kt):
                    for hi, (s, w) in enumerate(HALVES):
                        mi = nc.tensor.matmul(
                            ps7[:, s : s + w], W[:, k, 7 * 128 : 8 * 128],
                            rhs[:, k, s : s + w], start=(k == 0), stop=(k == kt - 1),
                        )
                        if hi > 0:
                            mi.ins.ldweights = False
                for hi, (s, w) in enumerate(HALVES):
                    nc.scalar.activation(
                        HRAW[:, 7, s : s + w], ps7[:, s : s + w], RELU,
                        accum_out=Sh[:, hi : hi + 1],
                    )
                for hi, (s, w) in enumerate(HALVES):
                    sq_tile(7, Sh, 2 + hi, s, w)
                nc.vector.tensor_tensor(S2[:, nf - 1 : nf], Sh[:, 0:1], Sh[:, 1:2], ADD)
                nc.vector.tensor_tensor(
                    S2[:, 2 * nf - 1 : 2 * nf], Sh[:, 2:3], Sh[:, 3:4], ADD
                )
                cc2 = ar_start(li, 9, S2, 2 * nf)
                A2, C2 = ar_finish(li, 9, finalj, cc2)
                for jj, jt in enumerate(finalj):
                    apply_tile(Hdst, jt, A2, C2, jj)

            # ---- layers ----
            mlp_layer(0, KT_IN, XT, W1, H)
            mlp_layer(1, JT, H, W2, H2)
            mlp_layer(2, JT, H2, W3, H)

            # ---- layer 4 (no relu/BN): k 0..3 run right at L3 end, k 4..7
            # after L3's second AllReduce applies ----
            pso = pspool.tile([128, BL], F32, tag="ps", name="ps_out")
            for k in range(JT):
                mm_pair(pso[0:CLSP, :], W4[:, k, :], H, k, k == 0, k == JT - 1)
            OUTS = spool.tile([CLSP, BL], F32, tag="OUTS")
            nc.scalar.copy(OUTS[:], pso[0:CLSP, :])
            nc.sync.dma_start(out_d[:], OUTS[:])
            # DCE anchor for the warmup AllGather (last gpsimd op; row 15 of
            # out_d is padding the host never reads back).
            nc.gpsimd.dma_start(out_d[CLSP - 1 : CLSP, 0:1], wu_out[0:1, :])

    nc.compile()
    return nc


def _get_nc():
    if "nc" not in _CACHE:
        _CACHE["nc"] = _build()
    return _CACHE["nc"]


def _prep_inputs(x, W1, W2, W3, W4, g1, b1, g2, b2, g3, b3):
    x2 = np.asarray(x, dtype=np.float32).reshape(B, KIN)
    xt = np.ascontiguousarray(x2.T).astype(ml_dtypes.bfloat16)  # [3072, 8192]

    def pmajor(a):
        kt = a.shape[0] // 128
        return np.ascontiguousarray(
            a.reshape(kt, 128, a.shape[1]).transpose(1, 0, 2)
        )

    def bin_t(w, pad=None):
        wb = np.where(np.asarray(w, dtype=np.float32) >= 0, 1.0, -1.0)
        wt = np.ascontiguousarray(wb.T).astype(ml_dtypes.bfloat16)  # [in, out]
        if pad is not None and wt.shape[1] < pad:
            wt = np.concatenate(
                [wt, np.zeros((wt.shape[0], pad - wt.shape[1]), wt.dtype)], axis=1
            )
        return pmajor(wt)

    w1t = bin_t(W1)
    w2t = bin_t(W2)
    w3t = bin_t(W3)
    w4t = bin_t(W4, pad=CLSP)

    bnp = np.zeros((128, 6 * JT), dtype=np.float32)
    for l, p in enumerate([g1, b1, g2, b2, g3, b3]):
        pa = np.asarray(p, dtype=np.float32)
        for jt in range(JT):
            bnp[:, l * JT + jt] = pa[jt * 128 : (jt + 1) * 128]

    shared = {"w1t": w1t, "w2t": w2t, "w3t": w3t, "w4t": w4t, "bnp": bnp}
    in_maps = []
    for c in range(NCORES):
        m = dict(shared)
        m["xt"] = pmajor(np.ascontiguousarray(xt[:, c * BL : (c + 1) * BL]))
        in_maps.append(m)
    return in_maps


def _run(inputs, trace=False):
    nc = _get_nc()
    in_maps = _prep_inputs(**inputs)
    res = bass_utils.run_bass_kernel_spmd(
        nc, in_maps, core_ids=list(range(NCORES)), trace=trace
    )
    out = np.empty((B, CLS), dtype=np.float32)
    for c in range(NCORES):
        out[c * BL : (c + 1) * BL, :] = res.results[c]["out"][:CLS, :].T
    return out, res


def kernel(**inputs):
    out, _ = _run(inputs, trace=False)
    return out


# revision 21
# speedup vs baseline: 1.0120x; 1.0120x over previous
"""Baseline nn_BinaryLinear kernel + PE-clock warmup + finer first feed chunks.

Identical to the originally staged kernel (3 staggered AllGathers per layer,
feature-tile groups 4+2+2, stats/gathers on the sync ring) except:
  - ~10 matmuls on zeroed scratch warm the PE clock (HAM) while the first
    XT/W1 chunks stream in, so the real stream starts at speed ~6us earlier.
  - The first feed chunks are single k-tiles.
"""
import os
import sys

for _p in ("/opt/trn_rl_repo",):
    if os.path.isdir(_p) and _p not in sys.path:
        sys.path.insert(0, _p)

import numpy as np
import ml_dtypes

from concourse import bacc, tile, mybir
from concourse import bass_utils

NCORES = 8
B = 8192
BL = B // NCORES
KIN = 3072
KT_IN = KIN // 128
HID = 1024
JT = HID // 128
CLS = 10
CLSP = 16
EPS = 1e-5
BF16 = mybir.dt.bfloat16
F32 = mybir.dt.float32
ADD = mybir.AluOpType.add
SUB = mybir.AluOpType.subtract
MUL = mybir.AluOpType.mult
RELU = mybir.ActivationFunctionType.Relu

_CACHE = {}


def _build(stage=99):
    nc = bacc.Bacc("TRN2", target_bir_lowering=False, debug=False, num_devices=NCORES)

    xt_d = nc.dram_tensor("xt", [128, KT_IN, BL], BF16, kind="ExternalInput")
    w1_d = nc.dram_tensor("w1t", [128, KT_IN, HID], BF16, kind="ExternalInput")
    w2_d = nc.dram_tensor("w2t", [128, JT, HID], BF16, kind="ExternalInput")
    w3_d = nc.dram_tensor("w3t", [128, JT, HID], BF16, kind="ExternalInput")
    w4_d = nc.dram_tensor("w4t", [128, JT, CLSP], BF16, kind="ExternalInput")
    bnp_d = nc.dram_tensor("bnp", [128, 6 * JT], F32, kind="ExternalInput")
    out_d = nc.dram_tensor("out", [CLSP, BL], F32, kind="ExternalOutput")

    nhalves = [(s, min(512, BL - s)) for s in range(0, BL, 512)]

    with tile.TileContext(nc) as tc:
        with (
            tc.tile_pool(name="weights", bufs=1) as wpool,
            tc.tile_pool(name="acts", bufs=1) as apool,
            tc.tile_pool(name="scratch", bufs=2) as scrpool,
            tc.tile_pool(name="stats", bufs=2) as spool,
            tc.tile_pool(name="psum", bufs=4, space="PSUM") as pspool,
            tc.tile_pool(name="dram", bufs=2, space="DRAM") as dpool,
        ):
            XT = wpool.tile([128, KT_IN, BL], BF16, tag="XT")
            W1 = wpool.tile([128, KT_IN, HID], BF16, tag="W1")
            W2 = wpool.tile([128, JT, HID], BF16, tag="W2")
            W3 = wpool.tile([128, JT, HID], BF16, tag="W3")
            W4 = wpool.tile([128, JT, CLSP], BF16, tag="W4")
            BNP = wpool.tile([128, 6 * JT], F32, tag="BNP")
            HRAW = apool.tile([128, JT, BL], BF16, tag="HRAW")
            H = apool.tile([128, JT, BL], BF16, tag="H")
            WZ = wpool.tile([128, 512], BF16, tag="WZ")

            # Warmup collective: absorbs the ncfw wake latency.
            wu_in = dpool.tile([128, 1], F32, tag="wu_in")
            wu_out = dpool.tile([NCORES * 128, 1], F32, tag="wu_out")
            nc.gpsimd.collective_compute(
                "AllGather",
                mybir.AluOpType.bypass,
                replica_groups=[list(range(NCORES))],
                ins=[wu_in.opt()],
                outs=[wu_out.opt()],
            )
            nc.gpsimd.memset(WZ[:], 0)
            # PE-clock warmup while the first feed chunks land.
            wps = pspool.tile([128, BL], F32, tag="ps", name="ps_warm")
            for i in range(10):
                mi = nc.tensor.matmul(
                    wps[:, 0:512], WZ[:, 0:128], WZ[:, 0:512],
                    start=True, stop=True,
                )
                if i > 0:
                    mi.ins.ldweights = False

            nc.sync.dma_start(BNP[:], bnp_d[:])
            feed = [1, 1, 2, 2, 4, 4, 4, 4, 2]
            c = 0
            for w in feed:
                w = min(w, KT_IN - c)
                if w <= 0:
                    break
                nc.sync.dma_start(XT[:, c : c + w, :], xt_d[:, c : c + w, :])
                nc.scalar.dma_start(W1[:, c : c + w, :], w1_d[:, c : c + w, :])
                c += w

            def mm_pair(ps, Wk, rhs, k, kt):
                for idx, (s, w) in enumerate(nhalves):
                    mi = nc.tensor.matmul(
                        ps[:, s : s + w],
                        Wk,
                        rhs[:, k, s : s + w],
                        start=(k == 0),
                        stop=(k == kt - 1),
                    )
                    if idx > 0:
                        mi.ins.ldweights = False

            def relu_square(ps, jt, S, j, n):
                nc.scalar.activation(
                    HRAW[:, jt, :], ps[:], RELU,
                    accum_out=S[:, j : j + 1],
                )
                scr = scrpool.tile([128, BL], BF16, tag="scr")
                nc.vector.scalar_tensor_tensor(
                    scr[:], HRAW[:, jt, :], 0.0, HRAW[:, jt, :],
                    mybir.AluOpType.bypass, MUL,
                    accum_out=S[:, n + j : n + j + 1],
                )

            def bn_ag_start(li, h, S, n):
                cc_in = dpool.tile([128, 2 * n], F32, tag="cc_in",
                                   name=f"cc_in_{li}_{h}")
                cc_out = dpool.tile([NCORES * 128, 2 * n], F32, tag="cc_out",
                                    name=f"cc_out_{li}_{h}")
                nc.sync.dma_start(cc_in[:], S[:])
                nc.gpsimd.collective_compute(
                    "AllGather",
                    mybir.AluOpType.bypass,
                    replica_groups=[list(range(NCORES))],
                    ins=[cc_in.opt()],
                    outs=[cc_out.opt()],
                )
                return cc_out

            def bn_finish(li, h, jts, cc_out, Hdst):
                n = len(jts)
                GAT = spool.tile([128, NCORES, 2 * n], F32, tag="GAT",
                                 name=f"GAT_{li}_{h}")
                nc.sync.dma_start(
                    GAT[:], cc_out.opt().rearrange("(c p) s -> p c s", p=128)
                )
                T4 = spool.tile([128, 4, 2 * n], F32, tag="T4", name=f"T4_{li}_{h}")
                nc.vector.tensor_tensor(T4[:], GAT[:, 0:4, :], GAT[:, 4:8, :], ADD)
                T2 = spool.tile([128, 2, 2 * n], F32, tag="T2", name=f"T2_{li}_{h}")
                nc.vector.tensor_tensor(T2[:], T4[:, 0:2, :], T4[:, 2:4, :], ADD)
                SS = spool.tile([128, 2 * n], F32, tag="SS", name=f"SS_{li}_{h}")
                nc.vector.tensor_tensor(SS[:], T2[:, 0, :], T2[:, 1, :], ADD)

                MEAN = spool.tile([128, n], F32, tag="MEAN", name=f"MEAN_{li}_{h}")
                nc.vector.tensor_scalar_mul(MEAN[:], SS[:, 0:n], 1.0 / B)
                VPE = spool.tile([128, n], F32, tag="VPE", name=f"VPE_{li}_{h}")
                nc.vector.tensor_scalar(
                    VPE[:], SS[:, n : 2 * n], 1.0 / B, EPS, MUL, ADD
                )
                MSQ = spool.tile([128, n], F32, tag="MSQ", name=f"MSQ_{li}_{h}")
                nc.vector.tensor_tensor(MSQ[:], MEAN[:], MEAN[:], MUL)
                VAR = spool.tile([128, n], F32, tag="VAR", name=f"VAR_{li}_{h}")
                nc.vector.tensor_tensor(VAR[:], VPE[:], MSQ[:], SUB)
                RINV = spool.tile([128, n], F32, tag="RINV", name=f"RINV_{li}_{h}")
                nc.vector.reciprocal(RINV[:], VAR[:])
                RSTD = spool.tile([128, n], F32, tag="RSTD", name=f"RSTD_{li}_{h}")
                nc.scalar.sqrt(RSTD[:], RINV[:])
                g0 = (2 * li) * JT + jts[0]
                b0 = (2 * li + 1) * JT + jts[0]
                A = spool.tile([128, n], F32, tag="A", name=f"A_{li}_{h}")
                nc.vector.tensor_tensor(A[:], RSTD[:], BNP[:, g0 : g0 + n], MUL)
                AM = spool.tile([128, n], F32, tag="AM", name=f"AM_{li}_{h}")
                nc.vector.tensor_tensor(AM[:], A[:], MEAN[:], MUL)
                C = spool.tile([128, n], F32, tag="C", name=f"C_{li}_{h}")
                nc.vector.tensor_tensor(C[:], BNP[:, b0 : b0 + n], AM[:], SUB)
                for j, jt in enumerate(jts):
                    nc.vector.tensor_scalar(
                        Hdst[:, jt, :],
                        HRAW[:, jt, :],
                        A[:, j : j + 1],
                        C[:, j : j + 1],
                        MUL,
                        ADD,
                    )

            def mlp_layer(li, kt, rhs, W, Hdst):
                g0 = list(range(min(4, JT)))
                groups = [g0]
                rest = list(range(len(g0), JT))
                for i in range(0, len(rest), 2):
                    groups.append(rest[i : i + 2])

                Ss, ccs = [], []

                def group_stats_done(gi):
                    ccs.append(bn_ag_start(li, gi, Ss[gi], len(groups[gi])))

                n0 = len(g0)
                S0 = spool.tile([128, 2 * n0], F32, tag="S_g0", name=f"S{li}_0")
                Ss.append(S0)
                pss = [
                    pspool.tile([128, BL], F32, tag="ps", name=f"ps_g{j}")
                    for j in range(n0)
                ]
                for k in range(kt):
                    for j in range(n0):
                        mm_pair(pss[j], W[:, k, j * 128 : (j + 1) * 128], rhs, k, kt)
                for j in range(n0):
                    relu_square(pss[j], j, S0, j, n0)
                group_stats_done(0)

                for gi, jts in enumerate(groups[1:], 1):
                    n = len(jts)
                    S = spool.tile(
                        [128, 2 * n], F32, tag=f"S_g{gi}", name=f"S{li}_{gi}"
                    )
                    Ss.append(S)
                    for i, jt in enumerate(jts):
                        ps = pspool.tile(
                            [128, BL], F32, tag="ps", name=f"ps_s{gi}_{i}"
                        )
                        for k in range(kt):
                            mm_pair(
                                ps, W[:, k, jt * 128 : (jt + 1) * 128], rhs, k, kt
                            )
                        relu_square(ps, jt, S, i, n)
                    bn_finish(li, gi - 1, groups[gi - 1], ccs[gi - 1], Hdst)
                    group_stats_done(gi)
                bn_finish(li, len(groups) - 1, groups[-1], ccs[-1], Hdst)

            H2 = apool.tile([128, JT, BL], BF16, tag="H2")
            mlp_layer(0, KT_IN, XT, W1, H)
            nc.sync.dma_start(W2[:], w2_d[:])
            mlp_layer(1, JT, H, W2, H2)
            nc.scalar.dma_start(W3[:], w3_d[:])
            mlp_layer(2, JT, H2, W3, H)
            nc.sync.dma_start(W4[:], w4_d[:])

            ps4 = pspool.tile([CLSP, BL], F32, tag="ps")
            for k in range(JT):
                mm_pair(ps4, W4[:, k, :], H, k, JT)
            OUTS = spool.tile([CLSP, BL], F32, tag="OUTS")
            nc.scalar.copy(OUTS[:], ps4[:])
            nc.sync.dma_start(out_d[:], OUTS[:])
            nc.gpsimd.dma_start(out_d[CLSP - 1 : CLSP, 0:1], wu_out[0:1, :])

    nc.compile()
    return nc


def _get_nc():
    if "nc" not in _CACHE:
        _CACHE["nc"] = _build()
    return _CACHE["nc"]


def _prep_inputs(x, W1, W2, W3, W4, g1, b1, g2, b2, g3, b3):
    x2 = np.asarray(x, dtype=np.float32).reshape(B, KIN)
    xt = np.ascontiguousarray(x2.T).astype(ml_dtypes.bfloat16)

    def pmajor(a):
        kt = a.shape[0] // 128
        return np.ascontiguousarray(
            a.reshape(kt, 128, a.shape[1]).transpose(1, 0, 2)
        )

    def bin_t(w, pad=None):
        wb = np.where(np.asarray(w, dtype=np.float32) >= 0, 1.0, -1.0)
        wt = np.ascontiguousarray(wb.T).astype(ml_dtypes.bfloat16)
        if pad is not None and wt.shape[1] < pad:
            wt = np.concatenate(
                [wt, np.zeros((wt.shape[0], pad - wt.shape[1]), wt.dtype)], axis=1
            )
        return pmajor(wt)

    w1t = bin_t(W1)
    w2t = bin_t(W2)
    w3t = bin_t(W3)
    w4t = bin_t(W4, pad=CLSP)

    bnp = np.zeros((128, 6 * JT), dtype=np.float32)
    for l, p in enumerate([g1, b1, g2, b2, g3, b3]):
        pa = np.asarray(p, dtype=np.float32)
        for jt in range(JT):
            bnp[:, l * JT + jt] = pa[jt * 128 : (jt + 1) * 128]

    shared = {"w1t": w1t, "w2t": w2t, "w3t": w3t, "w4t": w4t, "bnp": bnp}
    in_maps = []
    for c in range(NCORES):
        m = dict(shared)
        m["xt"] = pmajor(np.ascontiguousarray(xt[:, c * BL : (c + 1) * BL]))
        in_maps.append(m)
    return in_maps


def _run(inputs, trace=False):
    nc = _get_nc()
    in_maps = _prep_inputs(**inputs)
    res = bass_utils.run_bass_kernel_spmd(
        nc, in_maps, core_ids=list(range(NCORES)), trace=trace
    )
    out = np.empty((B, CLS), dtype=np.float32)
    for c in range(NCORES):
        out[c * BL : (c + 1) * BL, :] = res.results[c]["out"][:CLS, :].T
    return out, res


def kernel(**inputs):
    out, _ = _run(inputs, trace=False)
    return out
